# revision 5
# baseline (speedup 1.0000x reference)
"""Causal self-attention (RoPE, 16 heads, D=1024, B=2, T=2048) on 8 TRN2 NeuronCores.

Sharding: tensor-parallel over heads — 2 heads per core. Each core computes the
qkv projection for its heads (fp32r matmuls against host-pre-transposed x),
rotary embedding, causal attention in S^T layout (keys on PSUM partitions so
P^T = exp(S^T) feeds the attn@V matmul directly as the moving operand, with a
ones-column in V producing the softmax denominators on the tensor engine),
and a partial output projection against its slice of out_w rows. The host
sums the 8 partial projections and adds out_b.
"""

import os

import numpy as np

import concourse.mybir as mybir
import concourse.tile as tile
from concourse import bacc
from concourse.bass_utils import run_bass_kernel_spmd

F32 = mybir.dt.float32
F32R = mybir.dt.float32r
EXP = mybir.ActivationFunctionType.Exp

D = 1024
H = 16
HD = 64
B = 2
T = 2048
BT = B * T            # 4096
NCORES = 8
HLOC = H // NCORES    # 2 heads per core
NDC = D // 128        # 8 contraction chunks for the qkv projection
TBW = 256             # qkv token-block width
NTB = BT // TBW       # 16
NKB = T // 128        # 16 key blocks per (b, h)
NQ = T // 512         # 4 query super-blocks per (b, h)
SCALE = float(1.0 / np.sqrt(HD))


def build_nc():
    nc = bacc.Bacc("TRN2", debug=False)

    xT = nc.dram_tensor("xT", [D, BT], F32R, kind="ExternalInput")
    w = nc.dram_tensor("w", [D, 3 * HLOC * HD], F32R, kind="ExternalInput")
    bias = nc.dram_tensor("bias", [1, 3 * HLOC * HD], F32R, kind="ExternalInput")
    ones = nc.dram_tensor("ones", [1, TBW], F32R, kind="ExternalInput")
    ropeP = nc.dram_tensor("ropeP", [128, T], F32, kind="ExternalInput")
    ropeQ = nc.dram_tensor("ropeQ", [128, T], F32, kind="ExternalInput")
    tri = nc.dram_tensor("tri", [128, 128], F32R, kind="ExternalInput")
    zeros = nc.dram_tensor("zeros", [128, 384], F32R, kind="ExternalInput")
    ident = nc.dram_tensor("ident", [128, 64], F32, kind="ExternalInput")
    onescol = nc.dram_tensor("onescol", [128, NKB], F32R, kind="ExternalInput")
    wo = nc.dram_tensor("wo", [128, D], F32R, kind="ExternalInput")
    y = nc.dram_tensor("y", [BT, D], F32, kind="ExternalOutput")

    with tile.TileContext(nc) as tc:
        with tc.tile_pool(name="const", bufs=1) as const:
            w_sb = const.tile([128, NDC, 3 * HLOC * HD], F32R)
            nc.sync.dma_start(w_sb[:], w[:].rearrange("(dc p) f -> p dc f", p=128))
            b_sb = const.tile([1, 3 * HLOC * HD], F32R)
            nc.sync.dma_start(b_sb[:], bias[:])
            ones_sb = const.tile([1, TBW], F32R)
            nc.sync.dma_start(ones_sb[:], ones[:])
            P_sb = const.tile([128, T], F32)
            nc.sync.dma_start(P_sb[:], ropeP[:])
            Q_sb = const.tile([128, T], F32)
            nc.sync.dma_start(Q_sb[:], ropeQ[:])
            tri_sb = const.tile([128, 128], F32R)
            nc.sync.dma_start(tri_sb[:], tri[:])
            id_sb = const.tile([128, 64], F32)
            nc.sync.dma_start(id_sb[:], ident[:])
            oc_sb = const.tile([128, NKB], F32R)
            nc.sync.dma_start(oc_sb[:], onescol[:])
            wo_sb = const.tile([128, D], F32R)
            nc.sync.dma_start(wo_sb[:], wo[:])

            with tc.tile_pool(name="big", bufs=1) as big:
                qrot = big.tile([128, BT], F32R, tag="qrot")
                krot = big.tile([128, BT], F32R, tag="krot")
                vsb = [
                    big.tile([128, NKB, 65], F32R, name=f"v{i}", tag=f"v{i}")
                    for i in range(B * HLOC)
                ]

                # ---------------- Phase 1: QKV projection + rope + V transpose
                with (
                    tc.tile_pool(name="xt", bufs=2) as xt_pool,
                    tc.tile_pool(name="raw", bufs=3) as raw_pool,
                    tc.tile_pool(name="gsw", bufs=1) as gsw_pool,
                    tc.tile_pool(name="rtmp", bufs=2) as rtmp_pool,
                    tc.tile_pool(name="qkv_ps", bufs=3, space="PSUM") as qkv_psum,
                    tc.tile_pool(name="t_ps", bufs=2, space="PSUM") as t_psum,
                ):
                    qraw = raw_pool.tile([128, BT], F32R, tag="raw")
                    kraw = raw_pool.tile([128, BT], F32R, tag="raw")
                    vraw = raw_pool.tile([128, BT], F32R, tag="raw")
                    dests = (qraw, kraw, vraw)

                    for tb in range(NTB):
                        xt_t = xt_pool.tile([128, NDC, TBW], F32R, tag="xt")
                        nc.sync.dma_start(
                            xt_t[:],
                            xT[:, tb * TBW:(tb + 1) * TBW].rearrange(
                                "(dc p) t -> p dc t", p=128
                            ),
                        )
                        for ft in range(3):
                            ps = qkv_psum.tile([128, TBW], F32, tag="qkvps")
                            for dc in range(NDC):
                                nc.tensor.matmul(
                                    ps[:],
                                    w_sb[:, dc, ft * 128:(ft + 1) * 128],
                                    xt_t[:, dc, :],
                                    start=(dc == 0),
                                    stop=False,
                                )
                            nc.tensor.matmul(
                                ps[:],
                                b_sb[:, ft * 128:(ft + 1) * 128],
                                ones_sb[:],
                                start=False,
                                stop=True,
                            )
                            nc.scalar.copy(
                                dests[ft][:, tb * TBW:(tb + 1) * TBW], ps[:]
                            )

                    # rope: rot = raw * P + swap_halves(raw) * Q
                    for raw, rot in ((qraw, qrot), (kraw, krot)):
                        for b in range(B):
                            sl = slice(b * T, (b + 1) * T)
                            gsw = gsw_pool.tile([128, T], F32, tag="gsw")
                            for l in range(HLOC):
                                p0 = l * 64
                                nc.sync.dma_start(
                                    gsw[p0:p0 + 32, :],
                                    raw[p0 + 32:p0 + 64, sl].bitcast(F32),
                                )
                                nc.sync.dma_start(
                                    gsw[p0 + 32:p0 + 64, :],
                                    raw[p0:p0 + 32, sl].bitcast(F32),
                                )
                            t1 = rtmp_pool.tile([128, T], F32, tag="rt")
                            nc.vector.tensor_mul(t1[:], raw[:, sl].bitcast(F32), P_sb[:])
                            nc.vector.tensor_mul(gsw[:], gsw[:], Q_sb[:])
                            nc.vector.tensor_add(rot[:, sl], t1[:], gsw[:])

                    # V transpose: vraw [2h x 64, b*T+t] -> vsb[bh] [128 keys, kb, 64|1]
                    for b in range(B):
                        for l in range(HLOC):
                            bh = b * HLOC + l
                            nc.sync.dma_start(vsb[bh][:, :, 64], oc_sb[:])
                            for kb in range(NKB):
                                tp = t_psum.tile([128, 64], F32, tag="tps")
                                nc.tensor.transpose(
                                    tp[:],
                                    vraw[l * 64:(l + 1) * 64,
                                         b * T + kb * 128: b * T + (kb + 1) * 128
                                         ].bitcast(F32),
                                    id_sb[l * 64:(l + 1) * 64, :],
                                )
                                if kb % 2 == 0:
                                    nc.vector.tensor_copy(vsb[bh][:, kb, 0:64], tp[:])
                                else:
                                    nc.scalar.copy(vsb[bh][:, kb, 0:64], tp[:])

                # ---------------- Phase 2: attention + output projection
                with (
                    tc.tile_pool(name="p_sb", bufs=4) as p_pool,
                    tc.tile_pool(name="r_sb", bufs=2) as r_pool,
                    tc.tile_pool(name="rb_sb", bufs=2) as rb_pool,
                    tc.tile_pool(name="aTb", bufs=2) as aT_pool,
                    tc.tile_pool(name="y_sb", bufs=4) as y_pool,
                    tc.tile_pool(name="s_ps", bufs=4, space="PSUM") as s_psum,
                    tc.tile_pool(name="o_ps", bufs=2, space="PSUM") as o_psum,
                    tc.tile_pool(name="y_ps", bufs=2, space="PSUM") as y_psum,
                ):
                    for b in range(B):
                        for qb in range(NQ):
                            q0 = qb * 512
                            nkb = (q0 + 512) // 128
                            qsl = slice(b * T + q0, b * T + q0 + 512)
                            opss = [
                                o_psum.tile([65, 512], F32, name=f"ops{_l}", tag="ops")
                                for _l in range(HLOC)
                            ]
                            for kb in range(nkb):
                                k0 = kb * 128
                                r_off = kb - qb * 4  # >= 0: diagonal-region block
                                ksl = slice(b * T + k0, b * T + k0 + 128)
                                spss = []
                                for l in range(HLOC):
                                    p0 = l * 64
                                    sps = s_psum.tile([128, 512], F32, tag="sps")
                                    nc.tensor.matmul(
                                        sps[:],
                                        krot[p0:p0 + 64, ksl],
                                        qrot[p0:p0 + 64, qsl],
                                        start=True,
                                        stop=True,
                                    )
                                    spss.append(sps)
                                for l in range(HLOC):
                                    sps = spss[l]
                                    pt = p_pool.tile([128, 512], F32R, tag="pt")
                                    if r_off < 0:
                                        nc.scalar.activation(pt[:], sps[:], EXP, scale=SCALE)
                                    else:
                                        c0 = 128 * r_off
                                        if c0 > 0:
                                            nc.sync.dma_start(pt[:, 0:c0], zeros[:, 0:c0])
                                        nc.scalar.activation(
                                            pt[:, c0:512], sps[:, c0:512], EXP, scale=SCALE
                                        )
                                        nc.vector.tensor_mul(
                                            pt[:, c0:c0 + 128], pt[:, c0:c0 + 128], tri_sb[:]
                                        )
                                    nc.tensor.matmul(
                                        opss[l][:],
                                        vsb[b * HLOC + l][:, kb, :],
                                        pt[:],
                                        start=(kb == 0),
                                        stop=(kb == nkb - 1),
                                    )
                            aTb = aT_pool.tile([128, 512], F32R, tag="aTb")
                            for l in range(HLOC):
                                ops = opss[l]
                                r_sb = r_pool.tile([1, 512], F32, tag="r")
                                nc.vector.reciprocal(r_sb[:], ops[64:65, :])
                                rb_sb = rb_pool.tile([64, 512], F32, tag="rb")
                                nc.gpsimd.partition_broadcast(rb_sb[:], r_sb[:])
                                nc.vector.tensor_mul(
                                    aTb[l * 64:(l + 1) * 64, :], ops[0:64, :], rb_sb[:]
                                )
                            # partial output projection for these 512 tokens
                            for i in range(4):
                                row0 = b * T + q0 + i * 128
                                for nb in range(2):
                                    yps = y_psum.tile([128, 512], F32, tag="yps")
                                    nc.tensor.matmul(
                                        yps[:],
                                        aTb[:, i * 128:(i + 1) * 128],
                                        wo_sb[:, nb * 512:(nb + 1) * 512],
                                        start=True,
                                        stop=True,
                                    )
                                    ysb = y_pool.tile([128, 512], F32, tag="ysb")
                                    if (i * 2 + nb) % 2 == 0:
                                        nc.vector.tensor_copy(ysb[:], yps[:])
                                    else:
                                        nc.scalar.copy(ysb[:], yps[:])
                                    nc.sync.dma_start(
                                        y[row0:row0 + 128, nb * 512:(nb + 1) * 512],
                                        ysb[:],
                                    )

    nc.finalize()
    return nc


def _rope_tables():
    inv_freq = 1.0 / (10000.0 ** (np.arange(0, HD, 2, dtype=np.float32) / HD))
    t = np.arange(T, dtype=np.float32)
    freqs = t[:, None] * inv_freq[None, :]                          # [T, 32]
    rope = np.concatenate([np.sin(freqs), np.cos(freqs)], axis=-1)  # [T, 64]
    sin = rope[:, ::2]    # [T, 32]  (reference's "sin")
    cos = rope[:, 1::2]   # [T, 32]  (reference's "cos")
    # rot = raw * P + swap_halves(raw) * Q  with raw rows [x1(32) ; x2(32)]:
    #  rows 0..31  (out half0 = x1*cos - x2*sin; raw=x1, swap=x2): P=cos, Q=-sin
    #  rows 32..63 (out half1 = x1*sin + x2*cos; raw=x2, swap=x1): P=cos, Q=sin
    P64 = np.concatenate([cos.T, cos.T], axis=0)                    # [64, T]
    Q64 = np.concatenate([-sin.T, sin.T], axis=0)                   # [64, T]
    P128 = np.concatenate([P64, P64], axis=0).astype(np.float32)
    Q128 = np.concatenate([Q64, Q64], axis=0).astype(np.float32)
    return np.ascontiguousarray(P128), np.ascontiguousarray(Q128)


def make_core_inputs(x, qkv_w, qkv_b, out_w):
    """Build the per-core input maps for the 8-way head-parallel kernel."""
    x = np.asarray(x, dtype=np.float32)
    qkv_w = np.asarray(qkv_w, dtype=np.float32)
    qkv_b = np.asarray(qkv_b, dtype=np.float32)
    out_w = np.asarray(out_w, dtype=np.float32)

    xT = np.ascontiguousarray(x.reshape(BT, D).T)
    ropeP, ropeQ = _rope_tables()
    deint = np.concatenate([np.arange(0, HD, 2), np.arange(1, HD, 2)])  # [64]
    tri = np.triu(np.ones((128, 128), dtype=np.float32))  # valid: col >= row
    zeros = np.zeros((128, 384), dtype=np.float32)
    ident = np.concatenate([np.eye(64, dtype=np.float32)] * 2, axis=0)  # [128, 64]
    onescol = np.ones((128, NKB), dtype=np.float32)
    ones_tb = np.ones((1, TBW), dtype=np.float32)

    in_maps = []
    for c in range(NCORES):
        cols = []
        for sect, perm in ((0, deint), (1, deint), (2, np.arange(HD))):
            for l in range(HLOC):
                g = HLOC * c + l
                cols.append(sect * D + g * HD + perm)
        cols = np.concatenate(cols)
        w_core = np.ascontiguousarray(qkv_w[:, cols])
        b_core = np.ascontiguousarray(qkv_b[cols][None, :])
        wo_core = np.ascontiguousarray(out_w[c * 128:(c + 1) * 128, :])
        in_maps.append({
            "xT": xT,
            "w": w_core,
            "bias": b_core,
            "ones": ones_tb,
            "ropeP": ropeP,
            "ropeQ": ropeQ,
            "tri": tri,
            "zeros": zeros,
            "ident": ident,
            "onescol": onescol,
            "wo": wo_core,
        })
    return in_maps


_NC_CACHE = None


def kernel(x, qkv_w, qkv_b, out_w, out_b):
    global _NC_CACHE
    if _NC_CACHE is None:
        _NC_CACHE = build_nc()
    nc = _NC_CACHE
    in_maps = make_core_inputs(x, qkv_w, qkv_b, out_w)
    trace = bool(os.environ.get("ATTN_KERNEL_TRACE"))
    res = run_bass_kernel_spmd(
        nc, in_maps, core_ids=list(range(NCORES)), trace=trace,
    )
    kernel.last_results = res
    y = res.results[0]["y"].astype(np.float64)
    for c in range(1, NCORES):
        y = y + res.results[c]["y"].astype(np.float64)
    y = y + np.asarray(out_b, dtype=np.float64)[None, :]
    return np.ascontiguousarray(y.reshape(B, T, D).astype(np.float32))
